# revision 12
# baseline (speedup 1.0000x reference)
"""GATv2 layer (N=1024, IN=OUT=128, H=4, D=32) on 8 Trainium2 NeuronCores.

Sharding: row-block of the output/adjacency (128 rows of i per core);
node features (pre-transposed h^T, bf16) and projection weights replicated.

Math per core (rows i of this core's block), with leakyrelu(x) = x - 0.8*min(x,0)
= 0.6*x + 0.4*|x| and sl[i,h] = a.Wlh[i,h,:] (cancels in the softmax over j),
sr[j,h] = a.Wrh[j,h,:]:

  e[i,j,h] = c_i*sr[j,h] + m_i[:,j] @ blockdiag(s*a) - 100*(1-adj[i,j])

where per i-row either m_i = min(Wrh^T + Wlh_i, 0) produced on DVE
(tensor_scalar add+min, c=1, s=-0.8) or m_i = |Wrh^T + Wlh_i| produced on ACT
(Abs with per-partition bias, c=0.6, s=0.4).  Scores are O(3) so no
max-subtraction is needed, and the -100 mask term underflows exp to exact 0.

The i-loop is split into 4 phases of 32 rows.  Each phase accumulates its
scores into a quarter of one [128, 4096] f32 PSUM megatile (layout per phase:
8 j-tile regions of [j=128, 32i x 4h]).  The c*sr opener comes from one
matmul per (phase, jt) with wrhT as weights and a host-built Acol matrix as
rhs; the mask is added via an identity-expansion matmul from host-packed
maskb bf16.  When phase p's accumulation closes (mid-loop for p<3), ACT
exponentiates the whole quarter in ONE [128,1024] activation and the PE
aggregates agg-numerators + softmax denominators against V extended with a
ones-column, into accs views carved from phase 0's (already-read) PSUM space.
LayerNorm: bn_stats/bn_aggr + ACT Sqrt + DVE reciprocal, normalize+ReLU fused
into one ACT activation (Relu with per-partition scale/bias).
"""
import numpy as np
import ml_dtypes

import concourse.bacc as bacc
import concourse.tile as tile
from concourse import mybir
from concourse.bass_utils import run_bass_kernel_spmd

N = 1024
IN_DIM = 128
OUT_DIM = 128
H = 4
D = 32
NCORES = 8
BLK = N // NCORES   # 128 rows of i per core
NJT = 8             # j tiles of 128
NPH = 4             # i phases of 32
PHR = BLK // NPH    # 32 rows per phase
F32 = mybir.dt.float32
BF16 = mybir.dt.bfloat16
AF = mybir.ActivationFunctionType
ALU = mybir.AluOpType

N_ACT = 34  # rows produced on ACT (|.| form); rest on DVE (min form)
_ACT_SET = set(int(np.floor(k * BLK / N_ACT)) + 2 for k in range(N_ACT))
while len(_ACT_SET) < N_ACT:  # collisions paranoia
    _ACT_SET.add(max(_ACT_SET) - 1)


def _on_act(i):
    return i in _ACT_SET


DEBUG = False


def build_program(apply_affine=False):
    nc = bacc.Bacc(trn_type="TRN2", target_bir_lowering=False, debug=False,
                   num_devices=NCORES)

    def din(name, shape, dt):
        return nc.dram_tensor(name, shape, dt, kind="ExternalInput").ap()

    # packed bf16 inputs: critA gates stage 0, critB (hT second half) arrives
    # in parallel on the scalar-engine HWDGE queue
    critA_d = din("critA", [128, 3 * 128 + 512], BF16)  # wr|wl|hblkT|hT0
    critB_d = din("critB", [128, 512], BF16)            # hT1
    critb_d = din("critb", [128, 2 * H], BF16)          # Adve | Aact
    acol_d = din("acol", [128, NPH * PHR * H], BF16)    # c_i*a blockdiag per pair
    maskb_d = din("maskb", [PHR, NPH * N], BF16)        # (adj-1)*100, per-phase rows
    wv_d = din("wvb", [128, OUT_DIM], BF16)             # W_v bf16
    i32_d = din("i32rep", [PHR, PHR * H], BF16)         # repeat(I32, 4, axis=1)
    if apply_affine:
        gb_d = din("gbbc", [BLK, 2 * OUT_DIM], F32)     # gbc | bbc
    y_d = nc.dram_tensor("y", [BLK, OUT_DIM], F32, kind="ExternalOutput").ap()
    if DEBUG:
        dwrhT_d = nc.dram_tensor("dwrhT", [128, N], F32, kind="ExternalOutput").ap()
        dwT_d = nc.dram_tensor("dwT", [128, 4096], F32, kind="ExternalOutput").ap()
        dagg_d = nc.dram_tensor("dagg", [BLK, OUT_DIM], F32, kind="ExternalOutput").ap()
        dacc_d = nc.dram_tensor("dacc", [128, 132], F32, kind="ExternalOutput").ap()

    with tile.TileContext(nc) as tc:
        with tc.tile_pool(name="keep", bufs=1) as keep, \
             tc.tile_pool(name="small", bufs=4) as small, \
             tc.tile_pool(name="abs", bufs=24) as absp_pool, \
             tc.tile_pool(name="psall", bufs=1, space="PSUM") as psall:
            # --- DMA issues (scalar queue runs in parallel with sync) ---
            critB_sb = keep.tile([128, 512], BF16)
            nc.scalar.dma_start(out=critB_sb, in_=critB_d)
            critA_sb = keep.tile([128, 3 * 128 + 512], BF16)
            nc.sync.dma_start(out=critA_sb, in_=critA_d)
            critb_sb = keep.tile([128, 2 * H], BF16)
            nc.sync.dma_start(out=critb_sb, in_=critb_d)
            acol_sb = keep.tile([128, NPH * PHR * H], BF16)
            nc.gpsimd.dma_start(out=acol_sb, in_=acol_d)
            maskb_sb = keep.tile([PHR, NPH * N], BF16)
            nc.gpsimd.dma_start(out=maskb_sb, in_=maskb_d)
            wv_sb = keep.tile([128, OUT_DIM], BF16)
            nc.gpsimd.dma_start(out=wv_sb, in_=wv_d)
            i32_sb = keep.tile([PHR, PHR * H], BF16)
            nc.gpsimd.dma_start(out=i32_sb, in_=i32_d)
            if apply_affine:
                gb_sb = keep.tile([BLK, 2 * OUT_DIM], F32)
                nc.gpsimd.dma_start(out=gb_sb, in_=gb_d)

            wr_sb = critA_sb[:, 0:128]
            wl_sb = critA_sb[:, 128:256]
            hblkT_sb = critA_sb[:, 256:384]
            hT0_sb = critA_sb[:, 384:896]
            hT1_sb = critB_sb
            adve_sb = critb_sb[:, 0:H]
            aact_sb = critb_sb[:, H:2 * H]

            wrhT_sb = keep.tile([128, N], BF16)        # (h@W_r)^T  [hd, j]
            wlhT_sb = keep.tile([128, BLK], F32)       # (hblk@W_l)^T [hd, i]
            vext_sb = keep.tile([128, NJT * (D + 1) * H], BF16)
            wT_sb = keep.tile([128, NPH * PHR * H * NJT], BF16)  # exp scores
            agg_sb = keep.tile([BLK, OUT_DIM], F32)
            nc.gpsimd.memset(vext_sb, 1.0)

            # --- one PSUM megatile [128, 4096] = all 8 banks ---
            mega = psall.tile([128, 4096], F32)

            def bankreg(pp, jt):
                # pair pp in {0,1}: region [j=128, 64i x 4h] per j-tile
                return mega[:, 2048 * pp + 256 * jt:2048 * pp + 256 * (jt + 1)]

            # stage-0 scratch carved from late-phase PSUM space
            big0 = mega[:, 2048:2560]   # phase-2 space
            big1 = mega[:, 2560:3072]
            wp = mega[:, 1024:1152]     # phase-1 space
            # vp tiles for V projection live in phase-3 space
            vps = [mega[:, 3072 + 128 * jt:3072 + 128 * (jt + 1)]
                   for jt in range(NJT)]

            # --- stage 0: wrhT / wlhT (PE), evacuate on DVE+ACT ---
            nc.tensor.matmul(big0, wr_sb, hT0_sb, start=True, stop=True,
                             skip_group_check=True)
            nc.tensor.matmul(big1, wr_sb, hT1_sb, start=True, stop=True,
                             skip_group_check=True)
            nc.tensor.matmul(wp, wl_sb, hblkT_sb, start=True, stop=True,
                             skip_group_check=True)
            nc.vector.tensor_copy(wrhT_sb[:, 0:512], big0)
            nc.scalar.copy(wrhT_sb[:, 512:1024], big1)
            nc.scalar.copy(wlhT_sb, wp)

            # accs: numerators+denominator per head, carved per pair from
            # bank 0 (pair 0) / bank 1 (pair 1) after their exps read them.
            # NOTE: matmul start=True zeroes the ENTIRE 512-f32 PSUM bank, so
            # each bank must see exactly one start=True first per epoch.
            accsA = [mega[:, 33 * hh:33 * hh + 33] for hh in range(H)]
            accsB = [mega[:, 512 + 33 * hh:512 + 33 * hh + 33] for hh in range(H)]

            def emit_v(jt):
                # V projection for j-tile jt (PE) into phase-3 PSUM space
                hTs = (hT0_sb[:, jt * 128:(jt + 1) * 128] if jt < 4
                       else hT1_sb[:, (jt - 4) * 128:(jt - 3) * 128])
                nc.tensor.matmul(vps[jt], hTs, wv_sb, start=(jt % 4 == 0),
                                 stop=True, skip_group_check=True)

            def emit_vcopy(jt):
                base = jt * (D + 1) * H
                dst = vext_sb[:, base:base + (D + 1) * H].rearrange(
                    "p (h dd) -> p h dd", h=H)[:, :, 0:D]
                src = vps[jt].rearrange("p (h dd) -> p h dd", h=H)
                if jt % 2 == 0:
                    nc.scalar.copy(dst, src)
                else:
                    nc.vector.tensor_copy(dst, src)

            def emit_exp(p):
                # phase p = (pair pp, half q): strided [8 jt, 128] slices
                pp, q = p // 2, p % 2
                src = mega[:, 2048 * pp:2048 * (pp + 1)].rearrange(
                    "p (jt c) -> p jt c", jt=NJT)[:, :, 128 * q:128 * (q + 1)]
                dst = wT_sb[:, 2048 * pp:2048 * (pp + 1)].rearrange(
                    "p (jt c) -> p jt c", jt=NJT)[:, :, 128 * q:128 * (q + 1)]
                nc.scalar.activation(dst, src, AF.Exp)

            def emit_stage3(pp, k):
                # aggregation for pair pp (0: rows 0-63, 1: rows 64-127);
                # two matmuls per slot k (0..15), one per (jt, hh)
                for t in (2 * k, 2 * k + 1):
                    jt, hh = t // H, t % H
                    lhsT = wT_sb[:, 2048 * pp + 256 * jt + hh:
                                 2048 * pp + 256 * (jt + 1):H].opt()
                    rhs = vext_sb[:, jt * (D + 1) * H + hh * (D + 1):
                                  jt * (D + 1) * H + (hh + 1) * (D + 1)]
                    accs = accsA if pp == 0 else accsB
                    nc.tensor.matmul(accs[hh][64 * pp:64 * (pp + 1), :],
                                     lhsT, rhs, start=(jt == 0 and hh == 0),
                                     stop=(jt == NJT - 1),
                                     skip_group_check=True)

            # ------------- main loop: 4 phases x 32 rows -------------
            for i in range(BLK):
                p, il = i // PHR, i % PHR
                pp, ilg = i // 64, i % 64
                if ilg == 0:
                    # c*sr openers for this pair (rank-128 matmul vs Acol)
                    for jt in range(NJT):
                        nc.tensor.matmul(
                            bankreg(pp, jt),
                            wrhT_sb[:, jt * 128:(jt + 1) * 128],
                            acol_sb[:, 256 * pp:256 * (pp + 1)],
                            start=(jt % 2 == 0), stop=False,
                            skip_group_check=True)
                absp = absp_pool.tile([128, N], BF16, tag="absp")
                if _on_act(i):
                    nc.scalar.activation(absp, wrhT_sb, AF.Abs,
                                         bias=wlhT_sb[:, i:i + 1], scale=1.0)
                    arhs = aact_sb
                else:
                    nc.vector.tensor_scalar(absp, wrhT_sb,
                                            wlhT_sb[:, i:i + 1],
                                            0.0, ALU.add, ALU.min)
                    arhs = adve_sb
                for jt in range(NJT):
                    nc.tensor.matmul(
                        mega[:, 2048 * pp + 256 * jt + H * ilg:
                             2048 * pp + 256 * jt + H * ilg + H],
                        absp[:, jt * 128:(jt + 1) * 128], arhs,
                        start=False, stop=(ilg == 63),
                        skip_group_check=True)
                # mask matmuls for this phase, one per row mid-phase
                if 12 <= il < 12 + NJT:
                    jt = il - 12
                    q = p % 2
                    nc.tensor.matmul(
                        bankreg(pp, jt)[:, 128 * q:128 * (q + 1)],
                        maskb_sb[:, p * N + jt * 128:p * N + (jt + 1) * 128],
                        i32_sb, start=False, stop=False,
                        skip_group_check=True)
                # V projection interleaved into early phase-0 rows
                if p == 0:
                    if 2 <= il < 2 + NJT:
                        emit_v(il - 2)
                    if 3 <= il < 3 + NJT:
                        emit_vcopy(il - 3)
                # previous phase: exp (ACT); aggregation (PE) per pair
                if p >= 1 and il == 1:
                    emit_exp(p - 1)
                if p == 2 and 3 <= il < 19:
                    emit_stage3(0, il - 3)

            # ------------- tail: last phase exp + aggregation -------------
            emit_exp(NPH - 1)
            for k in range(16):
                emit_stage3(1, k)

            for pp, accs in ((0, accsA), (1, accsB)):
                lo, hi = 64 * pp, 64 * (pp + 1)
                for hh in range(H):
                    rinv = small.tile([BLK, 1], F32, tag="rinv")
                    nc.vector.reciprocal(rinv[lo:hi], accs[hh][lo:hi, D:D + 1])
                    nc.vector.tensor_scalar_mul(
                        agg_sb[lo:hi, hh * D:(hh + 1) * D],
                        accs[hh][lo:hi, 0:D], rinv[lo:hi])

            # ---------------- LayerNorm + ReLU ----------------
            stats = small.tile([BLK, 6], F32, tag="stats")
            nc.vector.bn_stats(out=stats, in_=agg_sb)
            mv = small.tile([BLK, 2], F32, tag="mv")
            nc.vector.bn_aggr(out=mv, in_=stats)
            veps = small.tile([BLK, 1], F32, tag="veps")
            nc.vector.tensor_scalar_add(veps, mv[:, 1:2], 1e-5)
            sdev = small.tile([BLK, 1], F32, tag="sdev")
            nc.scalar.activation(sdev, veps, AF.Sqrt)
            rstd = small.tile([BLK, 1], F32, tag="rstd")
            nc.vector.reciprocal(rstd, sdev)
            nmr = small.tile([BLK, 1], F32, tag="nmr")
            nc.vector.tensor_scalar(nmr, mv[:, 0:1], rstd, -1.0,
                                    ALU.mult, ALU.mult)
            yt = keep.tile([BLK, OUT_DIM], F32)
            if apply_affine:
                nc.vector.tensor_scalar(yt, agg_sb, rstd, nmr,
                                        ALU.mult, ALU.add)
                nc.vector.tensor_tensor(yt, yt, gb_sb[:, 0:OUT_DIM], ALU.mult)
                nc.vector.tensor_tensor(yt, yt, gb_sb[:, OUT_DIM:], ALU.add)
                nc.vector.tensor_scalar_max(yt, yt, 0.0)
            else:
                # relu((agg - mu) * rstd) fused into one ACT op
                nc.scalar.activation(yt, agg_sb, AF.Relu, bias=nmr, scale=rstd)
            nc.sync.dma_start(out=y_d, in_=yt)
            if DEBUG:
                dbg1 = keep.tile([128, N], F32)
                nc.vector.tensor_copy(dbg1, wrhT_sb)
                nc.sync.dma_start(out=dwrhT_d, in_=dbg1)
                dbg2 = keep.tile([128, 4096], F32)
                nc.vector.tensor_copy(dbg2, wT_sb)
                nc.sync.dma_start(out=dwT_d, in_=dbg2)
                nc.sync.dma_start(out=dagg_d, in_=agg_sb)
                dbg3 = keep.tile([128, 132], F32)
                nc.vector.tensor_copy(dbg3, mega[:, 0:132])
                nc.sync.dma_start(out=dacc_d, in_=dbg3)

    nc.compile()
    return nc


_NC = {}


def _get_program(apply_affine):
    if apply_affine not in _NC:
        _NC[apply_affine] = build_program(apply_affine)
    return _NC[apply_affine]


def _consts(a):
    bf = ml_dtypes.bfloat16
    a = np.asarray(a, np.float32)
    Adve = np.zeros((128, H), np.float32)
    Aact = np.zeros((128, H), np.float32)
    for hh in range(H):
        Adve[hh * D:(hh + 1) * D, hh] = -0.8 * a
        Aact[hh * D:(hh + 1) * D, hh] = 0.4 * a
    # Acol[hd, (pair, il, h')] = c_i * a[d] * [h == h']
    Acol = np.zeros((128, NPH * PHR * H), np.float32)
    for i in range(BLK):
        pp, ilg = i // 64, i % 64
        c = 0.6 if _on_act(i) else 1.0
        for hh in range(H):
            Acol[hh * D:(hh + 1) * D, pp * 256 + ilg * H + hh] = c * a
    I32rep = np.repeat(np.eye(PHR, dtype=np.float32), H, axis=1)
    return {
        "critb": np.ascontiguousarray(
            np.concatenate([Adve, Aact], axis=1)).astype(bf),
        "acol": np.ascontiguousarray(Acol).astype(bf),
        "i32rep": np.ascontiguousarray(I32rep).astype(bf),
    }


def kernel(h, adj, W_l, W_r, W_v, a, ln_g, ln_b, _trace=False, _tmpdir=None):
    bf = ml_dtypes.bfloat16
    affine = not (np.all(np.asarray(ln_g) == 1.0)
                  and np.all(np.asarray(ln_b) == 0.0))
    nc = _get_program(affine)
    h = np.asarray(h, np.float32)
    hT = np.ascontiguousarray(h.T).astype(bf)
    adjf = np.asarray(adj, np.float32)
    maskb = ((adjf - 1.0) * 100.0).astype(bf)
    consts = _consts(a)
    W_r = np.asarray(W_r, np.float32).astype(bf)
    W_l = np.asarray(W_l, np.float32).astype(bf)
    W_v = np.asarray(W_v, np.float32).astype(bf)
    base = {
        "critb": consts["critb"],
        "acol": consts["acol"],
        "i32rep": consts["i32rep"],
        "wvb": np.ascontiguousarray(W_v),
        "critB": np.ascontiguousarray(hT[:, 512:]),
    }
    if affine:
        base["gbbc"] = np.ascontiguousarray(np.concatenate(
            [np.tile(np.asarray(ln_g, np.float32)[None, :], (BLK, 1)),
             np.tile(np.asarray(ln_b, np.float32)[None, :], (BLK, 1))],
            axis=1))
    in_maps = []
    for c in range(NCORES):
        m = dict(base)
        m["critA"] = np.ascontiguousarray(np.concatenate(
            [W_r, W_l, hT[:, c * BLK:(c + 1) * BLK], hT[:, :512]], axis=1))
        mb = maskb[c * BLK:(c + 1) * BLK]  # [128, N] -> [32, NPH*N]
        m["maskb"] = np.ascontiguousarray(
            mb.reshape(NPH, PHR, N).transpose(1, 0, 2).reshape(PHR, NPH * N))
        in_maps.append(m)
    kw = {}
    if _trace:
        kw = dict(trace=True, tmpdir=_tmpdir)
    res = run_bass_kernel_spmd(nc, in_maps, list(range(NCORES)), **kw)
    y = np.concatenate([res.results[c]["y"] for c in range(NCORES)], axis=0)
    if DEBUG:
        return y, res
    if _trace:
        return y, res
    return y


# revision 13
# speedup vs baseline: 1.0213x; 1.0213x over previous
"""GATv2 layer (N=1024, IN=OUT=128, H=4, D=32) on 8 Trainium2 NeuronCores.

Sharding: row-block of the output/adjacency (128 rows of i per core);
node features (pre-transposed h^T, bf16) and projection weights replicated.

Math per core (rows i of this core's block), with leakyrelu(x) = x - 0.8*min(x,0)
= 0.6*x + 0.4*|x| and sl[i,h] = a.Wlh[i,h,:] (cancels in the softmax over j),
sr[j,h] = a.Wrh[j,h,:]:

  e[i,j,h] = c_i*sr[j,h] + m_i[:,j] @ blockdiag(s*a) - 100*(1-adj[i,j])

where per i-row either m_i = min(Wrh^T + Wlh_i, 0) produced on DVE
(tensor_scalar add+min, c=1, s=-0.8) or m_i = |Wrh^T + Wlh_i| produced on ACT
(Abs with per-partition bias, c=0.6, s=0.4).  Scores are O(3) so no
max-subtraction is needed, and the -100 mask term underflows exp to exact 0.

The i-loop is split into 4 phases of 32 rows.  Each phase accumulates its
scores into a quarter of one [128, 4096] f32 PSUM megatile (layout per phase:
8 j-tile regions of [j=128, 32i x 4h]).  The c*sr opener comes from one
matmul per (phase, jt) with wrhT as weights and a host-built Acol matrix as
rhs; the mask is added via an identity-expansion matmul from host-packed
maskb bf16.  When phase p's accumulation closes (mid-loop for p<3), ACT
exponentiates the whole quarter in ONE [128,1024] activation and the PE
aggregates agg-numerators + softmax denominators against V extended with a
ones-column, into accs views carved from phase 0's (already-read) PSUM space.
LayerNorm: bn_stats/bn_aggr + ACT Sqrt + DVE reciprocal, normalize+ReLU fused
into one ACT activation (Relu with per-partition scale/bias).
"""
import numpy as np
import ml_dtypes

import concourse.bacc as bacc
import concourse.tile as tile
from concourse import mybir
from concourse.bass_utils import run_bass_kernel_spmd

N = 1024
IN_DIM = 128
OUT_DIM = 128
H = 4
D = 32
NCORES = 8
BLK = N // NCORES   # 128 rows of i per core
NJT = 8             # j tiles of 128
NPH = 4             # i phases of 32
PHR = BLK // NPH    # 32 rows per phase
F32 = mybir.dt.float32
BF16 = mybir.dt.bfloat16
AF = mybir.ActivationFunctionType
ALU = mybir.AluOpType

N_ACT = 34  # rows produced on ACT (|.| form); rest on DVE (min form)
_ACT_SET = set(int(np.floor(k * BLK / N_ACT)) + 2 for k in range(N_ACT))
while len(_ACT_SET) < N_ACT:  # collisions paranoia
    _ACT_SET.add(max(_ACT_SET) - 1)


def _on_act(i):
    return i in _ACT_SET


DEBUG = False


def build_program(apply_affine=False):
    nc = bacc.Bacc(trn_type="TRN2", target_bir_lowering=False, debug=False,
                   num_devices=NCORES)

    def din(name, shape, dt):
        return nc.dram_tensor(name, shape, dt, kind="ExternalInput").ap()

    # packed bf16 inputs: critA gates stage 0, critB (hT second half) arrives
    # in parallel on the scalar-engine HWDGE queue
    critA_d = din("critA", [128, 3 * 128 + 512], BF16)  # wr|wl|hblkT|hT0
    critB_d = din("critB", [128, 512], BF16)            # hT1
    critb_d = din("critb", [128, 2 * H], BF16)          # Adve | Aact
    acol_d = din("acol", [128, NPH * PHR * H], BF16)    # c_i*a blockdiag per pair
    maskb_d = din("maskb", [64, 2 * N], BF16)           # (adj-1)*100, per-pair rows
    wv_d = din("wvb", [128, OUT_DIM], BF16)             # W_v bf16
    i64_d = din("i64rep", [64, 64 * H], BF16)           # repeat(I64, 4, axis=1)
    if apply_affine:
        gb_d = din("gbbc", [BLK, 2 * OUT_DIM], F32)     # gbc | bbc
    y_d = nc.dram_tensor("y", [BLK, OUT_DIM], F32, kind="ExternalOutput").ap()
    if DEBUG:
        dwrhT_d = nc.dram_tensor("dwrhT", [128, N], F32, kind="ExternalOutput").ap()
        dwT_d = nc.dram_tensor("dwT", [128, 4096], F32, kind="ExternalOutput").ap()
        dagg_d = nc.dram_tensor("dagg", [BLK, OUT_DIM], F32, kind="ExternalOutput").ap()
        dacc_d = nc.dram_tensor("dacc", [128, 132], F32, kind="ExternalOutput").ap()

    with tile.TileContext(nc) as tc:
        with tc.tile_pool(name="keep", bufs=1) as keep, \
             tc.tile_pool(name="small", bufs=4) as small, \
             tc.tile_pool(name="abs", bufs=24) as absp_pool, \
             tc.tile_pool(name="psall", bufs=1, space="PSUM") as psall:
            # --- DMA issues (scalar queue runs in parallel with sync) ---
            critB_sb = keep.tile([128, 512], BF16)
            nc.scalar.dma_start(out=critB_sb, in_=critB_d)
            critA_sb = keep.tile([128, 3 * 128 + 512], BF16)
            nc.sync.dma_start(out=critA_sb, in_=critA_d)
            critb_sb = keep.tile([128, 2 * H], BF16)
            nc.sync.dma_start(out=critb_sb, in_=critb_d)
            acol_sb = keep.tile([128, NPH * PHR * H], BF16)
            nc.gpsimd.dma_start(out=acol_sb, in_=acol_d)
            maskb_sb = keep.tile([64, 2 * N], BF16)
            nc.gpsimd.dma_start(out=maskb_sb, in_=maskb_d)
            wv_sb = keep.tile([128, OUT_DIM], BF16)
            nc.gpsimd.dma_start(out=wv_sb, in_=wv_d)
            i64_sb = keep.tile([64, 64 * H], BF16)
            nc.gpsimd.dma_start(out=i64_sb, in_=i64_d)
            if apply_affine:
                gb_sb = keep.tile([BLK, 2 * OUT_DIM], F32)
                nc.gpsimd.dma_start(out=gb_sb, in_=gb_d)

            wr_sb = critA_sb[:, 0:128]
            wl_sb = critA_sb[:, 128:256]
            hblkT_sb = critA_sb[:, 256:384]
            hT0_sb = critA_sb[:, 384:896]
            hT1_sb = critB_sb
            adve_sb = critb_sb[:, 0:H]
            aact_sb = critb_sb[:, H:2 * H]

            wrhT_sb = keep.tile([128, N], BF16)        # (h@W_r)^T  [hd, j]
            wlhT_sb = keep.tile([128, BLK], F32)       # (hblk@W_l)^T [hd, i]
            vext_sb = keep.tile([128, NJT * (D + 1) * H], BF16)
            wT_sb = keep.tile([128, NPH * PHR * H * NJT], BF16)  # exp scores
            agg_sb = keep.tile([BLK, OUT_DIM], F32)
            nc.gpsimd.memset(vext_sb, 1.0)

            # --- one PSUM megatile [128, 4096] = all 8 banks ---
            mega = psall.tile([128, 4096], F32)

            def bankreg(pp, jt):
                # pair pp in {0,1}: region [j=128, 64i x 4h] per j-tile
                return mega[:, 2048 * pp + 256 * jt:2048 * pp + 256 * (jt + 1)]

            # stage-0 scratch carved from late-phase PSUM space
            big0 = mega[:, 2048:2560]   # phase-2 space
            big1 = mega[:, 2560:3072]
            wp = mega[:, 1024:1152]     # phase-1 space
            # vp tiles for V projection live in phase-3 space
            vps = [mega[:, 3072 + 128 * jt:3072 + 128 * (jt + 1)]
                   for jt in range(NJT)]

            # --- stage 0: wrhT / wlhT (PE), evacuate on DVE+ACT ---
            nc.tensor.matmul(big0, wr_sb, hT0_sb, start=True, stop=True,
                             skip_group_check=True)
            nc.tensor.matmul(big1, wr_sb, hT1_sb, start=True, stop=True,
                             skip_group_check=True)
            nc.tensor.matmul(wp, wl_sb, hblkT_sb, start=True, stop=True,
                             skip_group_check=True)
            nc.vector.tensor_copy(wrhT_sb[:, 0:512], big0)
            nc.scalar.copy(wrhT_sb[:, 512:1024], big1)
            nc.scalar.copy(wlhT_sb, wp)

            # accs: numerators+denominator per head, carved per pair from
            # bank 0 (pair 0) / bank 1 (pair 1) after their exps read them.
            # NOTE: matmul start=True zeroes the ENTIRE 512-f32 PSUM bank, so
            # each bank must see exactly one start=True first per epoch.
            accsA = [mega[:, 33 * hh:33 * hh + 33] for hh in range(H)]
            accsB = [mega[:, 512 + 33 * hh:512 + 33 * hh + 33] for hh in range(H)]

            def emit_v(jt):
                # V projection for j-tile jt (PE) into phase-3 PSUM space
                hTs = (hT0_sb[:, jt * 128:(jt + 1) * 128] if jt < 4
                       else hT1_sb[:, (jt - 4) * 128:(jt - 3) * 128])
                nc.tensor.matmul(vps[jt], hTs, wv_sb, start=(jt % 4 == 0),
                                 stop=True, skip_group_check=True)

            def emit_vcopy(jt):
                base = jt * (D + 1) * H
                dst = vext_sb[:, base:base + (D + 1) * H].rearrange(
                    "p (h dd) -> p h dd", h=H)[:, :, 0:D]
                src = vps[jt].rearrange("p (h dd) -> p h dd", h=H)
                if jt % 2 == 0:
                    nc.scalar.copy(dst, src)
                else:
                    nc.vector.tensor_copy(dst, src)

            def emit_exp(p):
                # phase p = (pair pp, half q): strided [8 jt, 128] slices
                pp, q = p // 2, p % 2
                src = mega[:, 2048 * pp:2048 * (pp + 1)].rearrange(
                    "p (jt c) -> p jt c", jt=NJT)[:, :, 128 * q:128 * (q + 1)]
                dst = wT_sb[:, 2048 * pp:2048 * (pp + 1)].rearrange(
                    "p (jt c) -> p jt c", jt=NJT)[:, :, 128 * q:128 * (q + 1)]
                nc.scalar.activation(dst, src, AF.Exp)

            def emit_stage3(pp, k):
                # aggregation for pair pp (0: rows 0-63, 1: rows 64-127);
                # two matmuls per slot k (0..15), one per (jt, hh)
                for t in (2 * k, 2 * k + 1):
                    jt, hh = t // H, t % H
                    lhsT = wT_sb[:, 2048 * pp + 256 * jt + hh:
                                 2048 * pp + 256 * (jt + 1):H].opt()
                    rhs = vext_sb[:, jt * (D + 1) * H + hh * (D + 1):
                                  jt * (D + 1) * H + (hh + 1) * (D + 1)]
                    accs = accsA if pp == 0 else accsB
                    nc.tensor.matmul(accs[hh][64 * pp:64 * (pp + 1), :],
                                     lhsT, rhs, start=(jt == 0 and hh == 0),
                                     stop=(jt == NJT - 1),
                                     skip_group_check=True)

            # ------------- main loop: 4 phases x 32 rows -------------
            for i in range(BLK):
                p, il = i // PHR, i % PHR
                pp, ilg = i // 64, i % 64
                if ilg == 0:
                    # c*sr openers for this pair (rank-128 matmul vs Acol)
                    for jt in range(NJT):
                        nc.tensor.matmul(
                            bankreg(pp, jt),
                            wrhT_sb[:, jt * 128:(jt + 1) * 128],
                            acol_sb[:, 256 * pp:256 * (pp + 1)],
                            start=(jt % 2 == 0), stop=False,
                            skip_group_check=True)
                absp = absp_pool.tile([128, N], BF16, tag="absp")
                if _on_act(i):
                    nc.scalar.activation(absp, wrhT_sb, AF.Abs,
                                         bias=wlhT_sb[:, i:i + 1], scale=1.0)
                    arhs = aact_sb
                else:
                    nc.vector.tensor_scalar(absp, wrhT_sb,
                                            wlhT_sb[:, i:i + 1],
                                            0.0, ALU.add, ALU.min)
                    arhs = adve_sb
                for jt in range(NJT):
                    nc.tensor.matmul(
                        mega[:, 2048 * pp + 256 * jt + H * ilg:
                             2048 * pp + 256 * jt + H * ilg + H],
                        absp[:, jt * 128:(jt + 1) * 128], arhs,
                        start=False, stop=(ilg == 63),
                        skip_group_check=True)
                # mask matmuls, one per (pair, jt), during the pair's
                # first phase (covers both 32-row halves at once)
                if p % 2 == 0 and 12 <= il < 12 + NJT:
                    jt = il - 12
                    nc.tensor.matmul(
                        bankreg(pp, jt),
                        maskb_sb[:, pp * N + jt * 128:pp * N + (jt + 1) * 128],
                        i64_sb, start=False, stop=False,
                        skip_group_check=True)
                # V projection interleaved into early phase-0 rows
                if p == 0:
                    if 2 <= il < 2 + NJT:
                        emit_v(il - 2)
                    if 3 <= il < 3 + NJT:
                        emit_vcopy(il - 3)
                # previous phase: exp (ACT); aggregation (PE) per pair
                if p >= 1 and il == 6:
                    emit_exp(p - 1)
                if p == 2 and 8 <= il < 24:
                    emit_stage3(0, il - 8)

            # ------------- tail: last phase exp + aggregation -------------
            emit_exp(NPH - 1)
            for k in range(16):
                emit_stage3(1, k)

            rinv = small.tile([BLK, H], F32, tag="rinv")
            for pp, accs in ((0, accsA), (1, accsB)):
                lo, hi = 64 * pp, 64 * (pp + 1)
                base = 512 * pp
                dens = mega[lo:hi, base + D:base + D + H * (D + 1):D + 1]
                nc.vector.reciprocal(rinv[lo:hi], dens)
                for hh in range(H):
                    if hh % 2 == 0:
                        nc.vector.tensor_scalar_mul(
                            agg_sb[lo:hi, hh * D:(hh + 1) * D],
                            accs[hh][lo:hi, 0:D], rinv[lo:hi, hh:hh + 1])
                    else:
                        nc.scalar.activation(
                            agg_sb[lo:hi, hh * D:(hh + 1) * D],
                            accs[hh][lo:hi, 0:D], AF.Copy,
                            scale=rinv[lo:hi, hh:hh + 1])

            # ---------------- LayerNorm + ReLU ----------------
            stats = small.tile([BLK, 6], F32, tag="stats")
            nc.vector.bn_stats(out=stats, in_=agg_sb)
            mv = small.tile([BLK, 2], F32, tag="mv")
            nc.vector.bn_aggr(out=mv, in_=stats)
            veps = small.tile([BLK, 1], F32, tag="veps")
            nc.vector.tensor_scalar_add(veps, mv[:, 1:2], 1e-5)
            # rstd = 1/sqrt(veps): Quake + 2 Newton steps (avoids the Sqrt
            # activation, whose table set would evict Exp/Relu mid-kernel)
            I32T = mybir.dt.int32
            rstd = small.tile([BLK, 1], F32, tag="rstd")
            nc.vector.tensor_scalar(rstd.bitcast(I32T), veps.bitcast(I32T), 1,
                                    None, ALU.arith_shift_right)
            nc.vector.tensor_scalar(rstd.bitcast(I32T), rstd.bitcast(I32T), -1,
                                    0x5f3759df, ALU.mult, ALU.add)
            hv = small.tile([BLK, 1], F32, tag="hv")
            nc.vector.tensor_scalar_mul(hv, veps, -0.5)
            for _ in range(2):
                yy = small.tile([BLK, 1], F32, tag="yy")
                nc.vector.tensor_tensor(yy, rstd, rstd, ALU.mult)
                nc.vector.tensor_scalar(yy, yy, hv, 1.5, ALU.mult, ALU.add)
                nc.vector.tensor_tensor(rstd, rstd, yy, ALU.mult)
            nmr = small.tile([BLK, 1], F32, tag="nmr")
            nc.vector.tensor_scalar(nmr, mv[:, 0:1], rstd, -1.0,
                                    ALU.mult, ALU.mult)
            yt = keep.tile([BLK, OUT_DIM], F32)
            if apply_affine:
                nc.vector.tensor_scalar(yt, agg_sb, rstd, nmr,
                                        ALU.mult, ALU.add)
                nc.vector.tensor_tensor(yt, yt, gb_sb[:, 0:OUT_DIM], ALU.mult)
                nc.vector.tensor_tensor(yt, yt, gb_sb[:, OUT_DIM:], ALU.add)
                nc.vector.tensor_scalar_max(yt, yt, 0.0)
            else:
                # relu((agg - mu) * rstd) fused into one ACT op
                nc.scalar.activation(yt, agg_sb, AF.Relu, bias=nmr, scale=rstd)
            nc.sync.dma_start(out=y_d, in_=yt)
            if DEBUG:
                dbg1 = keep.tile([128, N], F32)
                nc.vector.tensor_copy(dbg1, wrhT_sb)
                nc.sync.dma_start(out=dwrhT_d, in_=dbg1)
                dbg2 = keep.tile([128, 4096], F32)
                nc.vector.tensor_copy(dbg2, wT_sb)
                nc.sync.dma_start(out=dwT_d, in_=dbg2)
                nc.sync.dma_start(out=dagg_d, in_=agg_sb)
                dbg3 = keep.tile([128, 132], F32)
                nc.vector.tensor_copy(dbg3, mega[:, 0:132])
                nc.sync.dma_start(out=dacc_d, in_=dbg3)

    nc.compile()
    return nc


_NC = {}


def _get_program(apply_affine):
    if apply_affine not in _NC:
        _NC[apply_affine] = build_program(apply_affine)
    return _NC[apply_affine]


def _consts(a):
    bf = ml_dtypes.bfloat16
    a = np.asarray(a, np.float32)
    Adve = np.zeros((128, H), np.float32)
    Aact = np.zeros((128, H), np.float32)
    for hh in range(H):
        Adve[hh * D:(hh + 1) * D, hh] = -0.8 * a
        Aact[hh * D:(hh + 1) * D, hh] = 0.4 * a
    # Acol[hd, (pair, il, h')] = c_i * a[d] * [h == h']
    Acol = np.zeros((128, NPH * PHR * H), np.float32)
    for i in range(BLK):
        pp, ilg = i // 64, i % 64
        c = 0.6 if _on_act(i) else 1.0
        for hh in range(H):
            Acol[hh * D:(hh + 1) * D, pp * 256 + ilg * H + hh] = c * a
    I64rep = np.repeat(np.eye(64, dtype=np.float32), H, axis=1)
    return {
        "critb": np.ascontiguousarray(
            np.concatenate([Adve, Aact], axis=1)).astype(bf),
        "acol": np.ascontiguousarray(Acol).astype(bf),
        "i64rep": np.ascontiguousarray(I64rep).astype(bf),
    }


def kernel(h, adj, W_l, W_r, W_v, a, ln_g, ln_b, _trace=False, _tmpdir=None):
    bf = ml_dtypes.bfloat16
    affine = not (np.all(np.asarray(ln_g) == 1.0)
                  and np.all(np.asarray(ln_b) == 0.0))
    nc = _get_program(affine)
    h = np.asarray(h, np.float32)
    hT = np.ascontiguousarray(h.T).astype(bf)
    adjf = np.asarray(adj, np.float32)
    maskb = ((adjf - 1.0) * 100.0).astype(bf)
    consts = _consts(a)
    W_r = np.asarray(W_r, np.float32).astype(bf)
    W_l = np.asarray(W_l, np.float32).astype(bf)
    W_v = np.asarray(W_v, np.float32).astype(bf)
    base = {
        "critb": consts["critb"],
        "acol": consts["acol"],
        "i64rep": consts["i64rep"],
        "wvb": np.ascontiguousarray(W_v),
        "critB": np.ascontiguousarray(hT[:, 512:]),
    }
    if affine:
        base["gbbc"] = np.ascontiguousarray(np.concatenate(
            [np.tile(np.asarray(ln_g, np.float32)[None, :], (BLK, 1)),
             np.tile(np.asarray(ln_b, np.float32)[None, :], (BLK, 1))],
            axis=1))
    in_maps = []
    for c in range(NCORES):
        m = dict(base)
        m["critA"] = np.ascontiguousarray(np.concatenate(
            [W_r, W_l, hT[:, c * BLK:(c + 1) * BLK], hT[:, :512]], axis=1))
        mb = maskb[c * BLK:(c + 1) * BLK]  # [128, N] -> [64, 2*N]
        m["maskb"] = np.ascontiguousarray(
            mb.reshape(2, 64, N).transpose(1, 0, 2).reshape(64, 2 * N))
        in_maps.append(m)
    kw = {}
    if _trace:
        kw = dict(trace=True, tmpdir=_tmpdir)
    res = run_bass_kernel_spmd(nc, in_maps, list(range(NCORES)), **kw)
    y = np.concatenate([res.results[c]["y"] for c in range(NCORES)], axis=0)
    if DEBUG:
        return y, res
    if _trace:
        return y, res
    return y
